# revision 36
# baseline (speedup 1.0000x reference)
"""Trainium2 Bass kernel for nn_BPBookLayer (retrieval_knn).

Computation (per full input):
  query = mean(x, axis=1)                         [B, D]
  scores = cos_sim(query, prototypes)             [B, P]
  top5 -> softmax -> agg = attn @ protos[top5]    [B, D]
  out = x + 0.1 * agg[:, None, :]

Sharding: data-parallel over batch B=32 across 8 cores (4 batches/core),
prototypes replicated.

Per-core implementation notes:
 - all DMA runs through gpsimd (SWDGE) casting DMAs: x and prototypes
   load f32 HBM -> fp16 SBUF (halving SBUF-side DMA cost), the result
   stores fp16 SBUF -> f32 HBM. Loads for all 4 batches are issued
   up-front (they fit SBUF in fp16), so the DMA ring never waits on
   compute; stores queue up behind them.
 - row-space dataflow keeps the PE instruction count low (the PE
   sequencer costs ~70-135ns/instruction and drives chain latency):
     q_row[1,D]    = ones_col.T @ x_tiles      (32 matmuls/batch, 512-wide)
     qT[128,8]     = 8 outer-products against one-hot rows
     scores[1,P]   = sum_dc qT[:,dc].T @ protoT[dc]   (16 matmuls)
     wtT[128,8]    = 8 outer-products of the masked softmax row
     agg[1,D]      = sum_pc wtT[:,pc].T @ proto[pc]   (16 matmuls)
 - top-5 is selection-free: per-half DVE max8 then a merge max8; mask
   scores >= t5 and multiply by exp in one scalar_tensor_tensor.
 - prototypes live in SBUF both raw fp16 [P, D] (for agg) and
   normalized-transposed fp16 [D, P] (for scores; built on-device with
   diag(1/||p||)-scaled transpose matmuls, 4 chunks batched per PSUM
   bank to cut the PSUM->SBUF copy count).
 - residual add in-place on DVE in fp16 (2x mode): xt += bc_fp16
   broadcast; softmax/norms stay f32.
"""

from contextlib import ExitStack

import numpy as np

import concourse.bacc as bacc
import concourse.bass as bass
import concourse.tile as tile
from concourse import mybir
from concourse.bass_utils import run_bass_kernel_spmd
from concourse.masks import make_identity

F32 = mybir.dt.float32
F16 = mybir.dt.float16
AF = mybir.ActivationFunctionType
ALU = mybir.AluOpType

B, L, D, P = 32, 2048, 1024, 1024
NCORES = 8
BLOC = B // NCORES  # batches per core
TROWS = 256  # L rows per x tile
TSUB = TROWS // 128
NT = L // TROWS     # x tiles per batch
DCH = D // 128      # d chunks
PCH = P // 128      # p chunks
XBUFS = BLOC * NT   # all x tiles resident
ALPHA = 0.1


def _kernel(tc, ctx, x, protos, out, repeat=1):
    nc = tc.nc

    singles = ctx.enter_context(tc.tile_pool(name="singles", bufs=1))
    xp = ctx.enter_context(tc.tile_pool(name="xp", bufs=XBUFS))
    sm = ctx.enter_context(tc.tile_pool(name="sm", bufs=2))
    # rotating <=2KB PSUM tiles (one bank each): proto transposes, q/score/agg
    # row halves, qT/wtT outer-products
    ps4 = ctx.enter_context(tc.tile_pool(name="ps4", bufs=2, space="PSUM"))
    psq = ctx.enter_context(tc.tile_pool(name="psq", bufs=2, space="PSUM"))
    ps_bc = ctx.enter_context(tc.tile_pool(name="ps_bc", bufs=2, space="PSUM"))

    for _rep in range(repeat):
        proto_sb = singles.tile([128, PCH, D], F16)
        protoT_sb = singles.tile([128, DCH, P], F16)
        inv_pnorm = singles.tile([128, PCH], F32)
        pnorm_sq = singles.tile([128, PCH], F32)

        # ---- every load up-front on the ring: protos alternate with batch-0
        # tiles (desc-gen overlaps transfers), then all remaining x.
        # Constants (gpsimd identity fill) are emitted a few DMA pairs in so
        # the Pool engine issues the first descriptors immediately. ----
        xt = [[None] * NT for _ in range(BLOC)]

        def load_tile(b, i):
            t_ = xp.tile([128, TSUB * D], F16, tag="x", name=f"x{b}_{i}")
            xt[b][i] = t_
            nc.gpsimd.dma_start(
                out=t_,
                in_=x[b, TROWS * i : TROWS * (i + 1), :].rearrange(
                    "(p t) d -> p (t d)", p=128
                ),
            )

        ident = singles.tile([128, 128], F32)
        e8 = singles.tile([1, 8, 8], F16)
        ones_col = singles.tile([128, 1], F16)
        ones_row = singles.tile([1, 128], F16)

        # the first NHEAD prototype rows ride HWDGE in f32: their transfer
        # fills the DMA head while the first gpsimd prep runs (HWDGE
        # dispatches ~400ns sooner than SWDGE); converted to fp16 by the
        # scalar engine
        NHEAD = 36
        proto0_f32 = singles.tile([NHEAD, D], F32)
        nc.sync.dma_start(out=proto0_f32, in_=protos[0:NHEAD, :])
        nc.scalar.copy(out=proto_sb[0:NHEAD, 0, :], in_=proto0_f32)

        # SWDGE ring: batch-0 tiles alternate with proto chunks, x tile first
        # in each pair — the 1456ns tile transfer exceeds the ~1038ns/DMA prep
        # rate, so the ring self-sustains from the first transfer, and protos
        # all land by ~20us so the chain for batch 0 starts early
        # chunk-0's remainder is short (523ns) — put it last so its negative
        # prep margin lands where the ring has slack
        proto_order = list(range(1, PCH)) + [0]
        for k in range(PCH):
            c = proto_order[k]
            load_tile(0, k)
            if c == 0:
                nc.gpsimd.dma_start(
                    out=proto_sb[NHEAD:128, 0, :],
                    in_=protos[NHEAD:128, :],
                )
            else:
                nc.gpsimd.dma_start(
                    out=proto_sb[:, c, :],
                    in_=protos[c * 128 : (c + 1) * 128, :],
                )
            if k == 5:
                # ---- constants, tucked behind the queued DMA preps ----
                make_identity(nc, ident)
                nc.vector.memset(e8, 0.0)
                for j in range(8):
                    nc.vector.memset(e8[0:1, j, j : j + 1], 1.0)
                nc.vector.memset(ones_col, 1.0)
                nc.vector.memset(ones_row, 1.0)
        q_sbs, qsqs = [], []

        def emit_q(b):
            # q rows: ones.T @ x, accumulated per 512-half across all tiles;
            # tile-sem gated, so this PE work rides along with the loads and
            # stays out of the per-batch drain chain
            ps_q = [
                psq.tile([1, 512], F32, tag="q", name=f"ps_q{b}_{h}")
                for h in range(2)
            ]
            for i in range(NT):
                for t in range(TSUB):
                    for h in range(2):
                        nc.tensor.matmul(
                            ps_q[h],
                            lhsT=ones_col,
                            rhs=xt[b][i][:, t * D + h * 512 : t * D + h * 512 + 512],
                            start=(i == 0 and t == 0),
                            stop=(i == NT - 1 and t == TSUB - 1),
                        )
            q_sb = sm.tile([1, D], F16, tag="q", bufs=BLOC, name=f"q_sb{b}")
            for h in range(2):
                nc.scalar.copy(out=q_sb[0:1, h * 512 : (h + 1) * 512], in_=ps_q[h])
            qsq_sc = sm.tile([1, 512], F32, tag="qsq_sc")
            qsq = sm.tile([1, 2], F32, tag="qsq", bufs=BLOC, name=f"qsq{b}")
            for h in range(2):
                nc.scalar.activation(
                    out=qsq_sc, in_=ps_q[h], func=AF.Square,
                    accum_out=qsq[0:1, h : h + 1],
                )
            q_sbs.append(q_sb)
            qsqs.append(qsq)

        emit_q(0)

        # ---- prototype norms + normalized transpose (off the load ring) ----
        sq_scratch = sm.tile([128, D], F32, tag="sqs", bufs=1)
        for c in proto_order:
            nc.scalar.activation(
                out=sq_scratch,
                in_=proto_sb[:, c, :],
                func=AF.Square,
                accum_out=pnorm_sq[:, c : c + 1],
            )
            nc.scalar.activation(
                out=inv_pnorm[:, c : c + 1], in_=pnorm_sq[:, c : c + 1], func=AF.Sqrt
            )
            nc.vector.reciprocal(
                out=inv_pnorm[:, c : c + 1], in_=inv_pnorm[:, c : c + 1]
            )
            # protoT_n[d, p] = proto[p, d] / ||proto_p||  via lhsT.T @ diag;
            # 4 transposed chunks share one PSUM bank (start zeroes the bank,
            # later chunks accumulate into their own zeroed columns)
            diag_c = sm.tile([128, 128], F16, tag="diag", bufs=2, name=f"diag_{c}")
            nc.vector.tensor_scalar_mul(diag_c, ident, inv_pnorm[:, c : c + 1])
            for half in range(2):
                pst = ps4.tile([128, 4, 128], F32, tag="ps")
                for j in range(4):
                    dc = half * 4 + j
                    nc.tensor.matmul(
                        pst[:, j, :],
                        lhsT=proto_sb[:, c, dc * 128 : (dc + 1) * 128],
                        rhs=diag_c,
                        start=(j == 0),
                        stop=(j == 3),
                        skip_group_check=True,
                    )
                dst = protoT_sb[:, half * 4 : half * 4 + 4, c * 128 : (c + 1) * 128]
                if half == 0:
                    nc.scalar.copy(out=dst, in_=pst)
                else:
                    nc.vector.tensor_copy(dst, pst)

        # ---- per batch chains (q already computed in the load phase) ----
        def emit_adds(b, bc_h):
            bc_b = bc_h.rearrange("p (o d) -> p o d", o=1).to_broadcast(
                [128, TSUB, D]
            )
            for i in range(NT):
                xv = xt[b][i].rearrange("p (t d) -> p t d", d=D)
                nc.vector.tensor_tensor(out=xv, in0=xv, in1=bc_b, op=ALU.add)
                nc.sync.dma_start(
                    out=out[b, TROWS * i : TROWS * (i + 1), :].rearrange(
                        "(p t) d -> p (t d)", p=128
                    ),
                    in_=xt[b][i],
                )

        pending = None

        def chain(b):
            nonlocal pending
            q_sb, qsq = q_sbs[b], qsqs[b]
            inv_qn = sm.tile([1, 1], F32, tag="inv_qn")
            nc.vector.tensor_add(inv_qn, qsq[0:1, 0:1], qsq[0:1, 1:2])
            nc.scalar.activation(out=inv_qn, in_=inv_qn, func=AF.Sqrt)
            nc.vector.reciprocal(out=inv_qn, in_=inv_qn)

            # qT[128, 8] via one-hot outer products (one PSUM group)
            ps_qt = ps4.tile([128, DCH], F32, tag="ps")
            for dc in range(DCH):
                nc.tensor.matmul(
                    ps_qt,
                    lhsT=q_sb[0:1, dc * 128 : (dc + 1) * 128],
                    rhs=e8[0:1, dc, :],
                    start=(dc == 0),
                    stop=(dc == DCH - 1),
                )
            qT_h = sm.tile([128, DCH], F16, tag="qTh")
            nc.scalar.copy(out=qT_h, in_=ps_qt)

            # scores row: sum_dc qT[:,dc].T @ protoT_n[dc]
            ps_s = [ps4.tile([1, 512], F32, tag="ps", name=f"ps_s{h}") for h in range(2)]
            for dc in range(DCH):
                for h in range(2):
                    nc.tensor.matmul(
                        ps_s[h],
                        lhsT=qT_h[:, dc : dc + 1],
                        rhs=protoT_sb[:, dc, h * 512 : (h + 1) * 512],
                        start=(dc == 0),
                        stop=(dc == DCH - 1),
                    )

            # top-8 per half, then merged top-8; t5 = 5th largest overall
            vals2 = sm.tile([1, 16], F32, tag="vals2")
            for h in range(2):
                nc.vector.max(out=vals2[0:1, 8 * h : 8 * h + 8], in_=ps_s[h])
            vals = sm.tile([1, 8], F32, tag="vals")
            nc.vector.max(out=vals, in_=vals2)

            # e = exp(scores / ||q||); den over the top-5; coef = 0.1/den
            e_row = sm.tile([1, P], F32, tag="erow")
            for h in range(2):
                nc.scalar.activation(
                    out=e_row[0:1, h * 512 : (h + 1) * 512],
                    in_=ps_s[h],
                    func=AF.Exp,
                    scale=inv_qn,
                )
            evals = sm.tile([1, 8], F32, tag="evals")
            nc.scalar.activation(out=evals, in_=vals, func=AF.Exp, scale=inv_qn)
            den = sm.tile([1, 1], F32, tag="den")
            nc.vector.reduce_sum(
                out=den, in_=evals[0:1, 0:5], axis=mybir.AxisListType.X
            )
            coef = sm.tile([1, 1], F32, tag="coef")
            nc.vector.reciprocal(out=coef, in_=den)
            nc.scalar.mul(out=coef, in_=coef, mul=ALPHA)

            # wt row = (scores >= t5) * e, fp16
            wt_h = sm.tile([1, P], F16, tag="wth")
            for h in range(2):
                nc.vector.scalar_tensor_tensor(
                    out=wt_h[0:1, h * 512 : (h + 1) * 512],
                    in0=ps_s[h],
                    scalar=vals[0:1, 4:5],
                    in1=e_row[0:1, h * 512 : (h + 1) * 512],
                    op0=ALU.is_ge,
                    op1=ALU.mult,
                )

            # wtT[128, 8] via one-hot outer products
            ps_wt = ps4.tile([128, PCH], F32, tag="ps")
            for pc in range(PCH):
                nc.tensor.matmul(
                    ps_wt,
                    lhsT=wt_h[0:1, pc * 128 : (pc + 1) * 128],
                    rhs=e8[0:1, pc, :],
                    start=(pc == 0),
                    stop=(pc == PCH - 1),
                )
            wtT_h = sm.tile([128, PCH], F16, tag="wtTh")
            nc.scalar.copy(out=wtT_h, in_=ps_wt)

            # agg row: sum_pc wtT[:,pc].T @ proto[pc], scaled into fp16
            # agg -> scale-copy -> broadcast -> fp16 copy, pipelined per
            # 512-half: half 0's Act/PE tail overlaps half 1's aggregation
            agg_h = sm.tile([1, D], F16, tag="agg")
            bc_ps = ps_bc.tile([128, D], F32, tag="bc")
            bc_h = sm.tile([128, D], F16, tag="bch")
            for h in range(2):
                ps_a = ps4.tile([1, 512], F32, tag="ps", name=f"ps_a{h}")
                for pc in range(PCH):
                    nc.tensor.matmul(
                        ps_a,
                        lhsT=wtT_h[:, pc : pc + 1],
                        rhs=proto_sb[:, pc, h * 512 : (h + 1) * 512],
                        start=(pc == 0),
                        stop=(pc == PCH - 1),
                    )
                nc.scalar.activation(
                    out=agg_h[0:1, h * 512 : (h + 1) * 512],
                    in_=ps_a,
                    func=AF.Copy,
                    scale=coef,
                )
                nc.tensor.matmul(
                    bc_ps[:, h * 512 : (h + 1) * 512],
                    lhsT=ones_row,
                    rhs=agg_h[0:1, h * 512 : (h + 1) * 512],
                    start=True,
                    stop=True,
                )
                nc.scalar.copy(
                    out=bc_h[:, h * 512 : (h + 1) * 512],
                    in_=bc_ps[:, h * 512 : (h + 1) * 512],
                )

            # previous batch's adds/stores AFTER this chain: the DVE
            # sequencer reaches the next chain's ops before the 8-add burst
            if pending is not None:
                emit_adds(*pending)
            pending = (b, bc_h)

        for b in range(1, BLOC):
            for i in range(NT):
                load_tile(b, i)
            chain(b - 1)
            emit_q(b)
        chain(BLOC - 1)
        emit_adds(*pending)


def build_nc(repeat=1):
    nc = bacc.Bacc("TRN2", target_bir_lowering=False)
    x = nc.dram_tensor("x", [BLOC, L, D], F32, kind="ExternalInput")
    protos = nc.dram_tensor("prototypes", [P, D], F32, kind="ExternalInput")
    # fp16 output buffer: the result is computed in fp16 anyway, so storing
    # fp16 halves HBM write traffic; the host upcasts after gathering
    out = nc.dram_tensor("out", [BLOC, L, D], F16, kind="ExternalOutput")
    with tile.TileContext(nc) as tc, ExitStack() as ctx:
        _kernel(tc, ctx, x[:], protos[:], out[:], repeat=repeat)
    nc.finalize()
    return nc


def kernel(x, prototypes):
    x = np.ascontiguousarray(x, dtype=np.float32)
    prototypes = np.ascontiguousarray(prototypes, dtype=np.float32)
    assert x.shape == (B, L, D) and prototypes.shape == (P, D)
    nc = build_nc()
    in_maps = [
        {"x": x[c * BLOC : (c + 1) * BLOC], "prototypes": prototypes}
        for c in range(NCORES)
    ]
    res = run_bass_kernel_spmd(nc, in_maps, core_ids=list(range(NCORES)))
    full = np.concatenate([r["out"] for r in res.results], axis=0)
    return full.astype(np.float32)


# revision 39
# speedup vs baseline: 1.0032x; 1.0032x over previous
"""Trainium2 Bass kernel for nn_BPBookLayer (retrieval_knn).

Computation (per full input):
  query = mean(x, axis=1)                         [B, D]
  scores = cos_sim(query, prototypes)             [B, P]
  top5 -> softmax -> agg = attn @ protos[top5]    [B, D]
  out = x + 0.1 * agg[:, None, :]

Sharding: data-parallel over batch B=32 across 8 cores (4 batches/core),
prototypes replicated.

Per-core implementation notes:
 - all DMA runs through gpsimd (SWDGE) casting DMAs: x and prototypes
   load f32 HBM -> fp16 SBUF (halving SBUF-side DMA cost), the result
   stores fp16 SBUF -> f32 HBM. Loads for all 4 batches are issued
   up-front (they fit SBUF in fp16), so the DMA ring never waits on
   compute; stores queue up behind them.
 - row-space dataflow keeps the PE instruction count low (the PE
   sequencer costs ~70-135ns/instruction and drives chain latency):
     q_row[1,D]    = ones_col.T @ x_tiles      (32 matmuls/batch, 512-wide)
     qT[128,8]     = 8 outer-products against one-hot rows
     scores[1,P]   = sum_dc qT[:,dc].T @ protoT[dc]   (16 matmuls)
     wtT[128,8]    = 8 outer-products of the masked softmax row
     agg[1,D]      = sum_pc wtT[:,pc].T @ proto[pc]   (16 matmuls)
 - top-5 is selection-free: per-half DVE max8 then a merge max8; mask
   scores >= t5 and multiply by exp in one scalar_tensor_tensor.
 - prototypes live in SBUF both raw fp16 [P, D] (for agg) and
   normalized-transposed fp16 [D, P] (for scores; built on-device with
   diag(1/||p||)-scaled transpose matmuls, 4 chunks batched per PSUM
   bank to cut the PSUM->SBUF copy count).
 - residual add in-place on DVE in fp16 (2x mode): xt += bc_fp16
   broadcast; softmax/norms stay f32.
"""

from contextlib import ExitStack

import numpy as np

import concourse.bacc as bacc
import concourse.bass as bass
import concourse.tile as tile
from concourse import mybir
from concourse.bass_utils import run_bass_kernel_spmd
from concourse.masks import make_identity

F32 = mybir.dt.float32
F16 = mybir.dt.float16
AF = mybir.ActivationFunctionType
ALU = mybir.AluOpType

B, L, D, P = 32, 2048, 1024, 1024
NCORES = 8
BLOC = B // NCORES  # batches per core
TROWS = 256  # L rows per x tile
TSUB = TROWS // 128
NT = L // TROWS     # x tiles per batch
DCH = D // 128      # d chunks
PCH = P // 128      # p chunks
XBUFS = BLOC * NT   # all x tiles resident
ALPHA = 0.1


def _kernel(tc, ctx, x, protos, out, repeat=1):
    nc = tc.nc

    singles = ctx.enter_context(tc.tile_pool(name="singles", bufs=1))
    xp = ctx.enter_context(tc.tile_pool(name="xp", bufs=XBUFS))
    sm = ctx.enter_context(tc.tile_pool(name="sm", bufs=2))
    # rotating <=2KB PSUM tiles (one bank each): proto transposes, q/score/agg
    # row halves, qT/wtT outer-products
    ps4 = ctx.enter_context(tc.tile_pool(name="ps4", bufs=2, space="PSUM"))
    psq = ctx.enter_context(tc.tile_pool(name="psq", bufs=2, space="PSUM"))
    ps_bc = ctx.enter_context(tc.tile_pool(name="ps_bc", bufs=2, space="PSUM"))

    for _rep in range(repeat):
        proto_sb = singles.tile([128, PCH, D], F16)
        protoT_sb = singles.tile([128, DCH, P], F16)
        inv_pnorm = singles.tile([128, PCH], F32)
        pnorm_sq = singles.tile([128, PCH], F32)

        # ---- every load up-front on the ring: protos alternate with batch-0
        # tiles (desc-gen overlaps transfers), then all remaining x.
        # Constants (gpsimd identity fill) are emitted a few DMA pairs in so
        # the Pool engine issues the first descriptors immediately. ----
        xt = [[None] * NT for _ in range(BLOC)]

        def load_tile(b, i):
            t_ = xp.tile([128, TSUB * D], F16, tag="x", name=f"x{b}_{i}")
            xt[b][i] = t_
            nc.gpsimd.dma_start(
                out=t_,
                in_=x[b, TROWS * i : TROWS * (i + 1), :].rearrange(
                    "(p t) d -> p (t d)", p=128
                ),
            )

        ident = singles.tile([128, 128], F32)
        e8 = singles.tile([1, 8, 8], F16)
        ones_col = singles.tile([128, 1], F16)
        ones_row = singles.tile([1, 128], F16)

        # the first NHEAD prototype rows ride HWDGE in f32: their transfer
        # fills the DMA head while the first gpsimd prep runs (HWDGE
        # dispatches ~400ns sooner than SWDGE); converted to fp16 by the
        # scalar engine
        NHEAD = 36
        proto0_f32 = singles.tile([NHEAD, D], F32)
        nc.sync.dma_start(out=proto0_f32, in_=protos[0:NHEAD, :])
        nc.scalar.copy(out=proto_sb[0:NHEAD, 0, :], in_=proto0_f32)

        # SWDGE ring: batch-0 tiles alternate with proto chunks, x tile first
        # in each pair — the 1456ns tile transfer exceeds the ~1038ns/DMA prep
        # rate, so the ring self-sustains from the first transfer, and protos
        # all land by ~20us so the chain for batch 0 starts early
        # chunk-0's remainder is short (523ns) — put it last so its negative
        # prep margin lands where the ring has slack
        proto_order = list(range(1, PCH)) + [0]
        for k in range(PCH):
            c = proto_order[k]
            load_tile(0, k)
            if c == 0:
                nc.gpsimd.dma_start(
                    out=proto_sb[NHEAD:128, 0, :],
                    in_=protos[NHEAD:128, :],
                )
            else:
                nc.gpsimd.dma_start(
                    out=proto_sb[:, c, :],
                    in_=protos[c * 128 : (c + 1) * 128, :],
                )
            if k == 5:
                # ---- constants, tucked behind the queued DMA preps ----
                make_identity(nc, ident)
                nc.vector.memset(e8, 0.0)
                for j in range(8):
                    nc.vector.memset(e8[0:1, j, j : j + 1], 1.0)
                nc.vector.memset(ones_col, 1.0)
                nc.vector.memset(ones_row, 1.0)
        q_sbs, qsqs = [], []

        def emit_q(b):
            # q rows: ones.T @ x, accumulated per 512-half across all tiles;
            # tile-sem gated, so this PE work rides along with the loads and
            # stays out of the per-batch drain chain
            ps_q = [
                psq.tile([1, 512], F32, tag="q", name=f"ps_q{b}_{h}")
                for h in range(2)
            ]
            for i in range(NT):
                for t in range(TSUB):
                    for h in range(2):
                        nc.tensor.matmul(
                            ps_q[h],
                            lhsT=ones_col,
                            rhs=xt[b][i][:, t * D + h * 512 : t * D + h * 512 + 512],
                            start=(i == 0 and t == 0),
                            stop=(i == NT - 1 and t == TSUB - 1),
                        )
            q_sb = sm.tile([1, D], F16, tag="q", bufs=BLOC, name=f"q_sb{b}")
            for h in range(2):
                nc.scalar.copy(out=q_sb[0:1, h * 512 : (h + 1) * 512], in_=ps_q[h])
            qsq_sc = sm.tile([1, 512], F32, tag="qsq_sc")
            qsq = sm.tile([1, 2], F32, tag="qsq", bufs=BLOC, name=f"qsq{b}")
            for h in range(2):
                nc.scalar.activation(
                    out=qsq_sc, in_=ps_q[h], func=AF.Square,
                    accum_out=qsq[0:1, h : h + 1],
                )
            q_sbs.append(q_sb)
            qsqs.append(qsq)

        emit_q(0)

        # ---- prototype norms + normalized transpose (off the load ring) ----
        sq_scratch = sm.tile([128, D], F32, tag="sqs", bufs=1)
        for c in proto_order:
            nc.scalar.activation(
                out=sq_scratch,
                in_=proto_sb[:, c, :],
                func=AF.Square,
                accum_out=pnorm_sq[:, c : c + 1],
            )
            nc.scalar.activation(
                out=inv_pnorm[:, c : c + 1], in_=pnorm_sq[:, c : c + 1], func=AF.Sqrt
            )
            nc.vector.reciprocal(
                out=inv_pnorm[:, c : c + 1], in_=inv_pnorm[:, c : c + 1]
            )
            # protoT_n[d, p] = proto[p, d] / ||proto_p||  via lhsT.T @ diag;
            # 4 transposed chunks share one PSUM bank (start zeroes the bank,
            # later chunks accumulate into their own zeroed columns)
            diag_c = sm.tile([128, 128], F16, tag="diag", bufs=2, name=f"diag_{c}")
            nc.vector.tensor_scalar_mul(diag_c, ident, inv_pnorm[:, c : c + 1])
            for half in range(2):
                pst = ps4.tile([128, 4, 128], F32, tag="ps")
                for j in range(4):
                    dc = half * 4 + j
                    nc.tensor.matmul(
                        pst[:, j, :],
                        lhsT=proto_sb[:, c, dc * 128 : (dc + 1) * 128],
                        rhs=diag_c,
                        start=(j == 0),
                        stop=(j == 3),
                        skip_group_check=True,
                    )
                dst = protoT_sb[:, half * 4 : half * 4 + 4, c * 128 : (c + 1) * 128]
                if half == 0:
                    nc.scalar.copy(out=dst, in_=pst)
                else:
                    nc.vector.tensor_copy(dst, pst)

        # ---- per batch chains (q already computed in the load phase) ----
        def emit_adds(b, bc_h):
            bc_b = bc_h.rearrange("p (o d) -> p o d", o=1).to_broadcast(
                [128, TSUB, D]
            )
            bc_1 = bc_h.rearrange("p (o d) -> p o d", o=1).to_broadcast(
                [128, 1, D]
            )
            for i in range(NT):
                xv = xt[b][i].rearrange("p (t d) -> p t d", d=D)
                if i == 0:
                    # first tile split in half: its first store launches
                    # ~500ns sooner, shrinking every batch-boundary DMA gap
                    for t in range(TSUB):
                        nc.vector.tensor_tensor(
                            out=xv[:, t : t + 1, :], in0=xv[:, t : t + 1, :],
                            in1=bc_1, op=ALU.add,
                        )
                        nc.sync.dma_start(
                            out=out[b, TROWS * i : TROWS * (i + 1), :].rearrange(
                                "(p t) d -> p t d", p=128
                            )[:, t, :],
                            in_=xt[b][i][:, t * D : (t + 1) * D],
                        )
                    continue
                nc.vector.tensor_tensor(out=xv, in0=xv, in1=bc_b, op=ALU.add)
                nc.sync.dma_start(
                    out=out[b, TROWS * i : TROWS * (i + 1), :].rearrange(
                        "(p t) d -> p (t d)", p=128
                    ),
                    in_=xt[b][i],
                )

        pending = None

        def chain(b):
            nonlocal pending
            q_sb, qsq = q_sbs[b], qsqs[b]
            inv_qn = sm.tile([1, 1], F32, tag="inv_qn")
            nc.vector.tensor_add(inv_qn, qsq[0:1, 0:1], qsq[0:1, 1:2])
            nc.scalar.activation(out=inv_qn, in_=inv_qn, func=AF.Sqrt)
            nc.vector.reciprocal(out=inv_qn, in_=inv_qn)

            # qT[128, 8] via one-hot outer products (one PSUM group)
            ps_qt = ps4.tile([128, DCH], F32, tag="ps")
            for dc in range(DCH):
                nc.tensor.matmul(
                    ps_qt,
                    lhsT=q_sb[0:1, dc * 128 : (dc + 1) * 128],
                    rhs=e8[0:1, dc, :],
                    start=(dc == 0),
                    stop=(dc == DCH - 1),
                )
            qT_h = sm.tile([128, DCH], F16, tag="qTh")
            nc.scalar.copy(out=qT_h, in_=ps_qt)

            # scores row: sum_dc qT[:,dc].T @ protoT_n[dc]
            ps_s = [ps4.tile([1, 512], F32, tag="ps", name=f"ps_s{h}") for h in range(2)]
            for dc in range(DCH):
                for h in range(2):
                    nc.tensor.matmul(
                        ps_s[h],
                        lhsT=qT_h[:, dc : dc + 1],
                        rhs=protoT_sb[:, dc, h * 512 : (h + 1) * 512],
                        start=(dc == 0),
                        stop=(dc == DCH - 1),
                    )

            # top-8 per half, then merged top-8; t5 = 5th largest overall
            vals2 = sm.tile([1, 16], F32, tag="vals2")
            for h in range(2):
                nc.vector.max(out=vals2[0:1, 8 * h : 8 * h + 8], in_=ps_s[h])
            vals = sm.tile([1, 8], F32, tag="vals")
            nc.vector.max(out=vals, in_=vals2)

            # e = exp(scores / ||q||); den over the top-5; coef = 0.1/den
            e_row = sm.tile([1, P], F32, tag="erow")
            for h in range(2):
                nc.scalar.activation(
                    out=e_row[0:1, h * 512 : (h + 1) * 512],
                    in_=ps_s[h],
                    func=AF.Exp,
                    scale=inv_qn,
                )
            evals = sm.tile([1, 8], F32, tag="evals")
            nc.scalar.activation(out=evals, in_=vals, func=AF.Exp, scale=inv_qn)
            den = sm.tile([1, 1], F32, tag="den")
            nc.vector.reduce_sum(
                out=den, in_=evals[0:1, 0:5], axis=mybir.AxisListType.X
            )
            coef = sm.tile([1, 1], F32, tag="coef")
            nc.vector.reciprocal(out=coef, in_=den)
            nc.scalar.mul(out=coef, in_=coef, mul=ALPHA)

            # wt row = (scores >= t5) * e, fp16
            wt_h = sm.tile([1, P], F16, tag="wth")
            for h in range(2):
                nc.vector.scalar_tensor_tensor(
                    out=wt_h[0:1, h * 512 : (h + 1) * 512],
                    in0=ps_s[h],
                    scalar=vals[0:1, 4:5],
                    in1=e_row[0:1, h * 512 : (h + 1) * 512],
                    op0=ALU.is_ge,
                    op1=ALU.mult,
                )

            # wtT[128, 8] via one-hot outer products
            ps_wt = ps4.tile([128, PCH], F32, tag="ps")
            for pc in range(PCH):
                nc.tensor.matmul(
                    ps_wt,
                    lhsT=wt_h[0:1, pc * 128 : (pc + 1) * 128],
                    rhs=e8[0:1, pc, :],
                    start=(pc == 0),
                    stop=(pc == PCH - 1),
                )
            wtT_h = sm.tile([128, PCH], F16, tag="wtTh")
            nc.scalar.copy(out=wtT_h, in_=ps_wt)

            # agg row: sum_pc wtT[:,pc].T @ proto[pc], scaled into fp16
            # agg -> scale-copy -> broadcast -> fp16 copy, pipelined per
            # 512-half: half 0's Act/PE tail overlaps half 1's aggregation
            agg_h = sm.tile([1, D], F16, tag="agg")
            bc_ps = ps_bc.tile([128, D], F32, tag="bc")
            bc_h = sm.tile([128, D], F16, tag="bch")
            for h in range(2):
                ps_a = ps4.tile([1, 512], F32, tag="ps", name=f"ps_a{h}")
                for pc in range(PCH):
                    nc.tensor.matmul(
                        ps_a,
                        lhsT=wtT_h[:, pc : pc + 1],
                        rhs=proto_sb[:, pc, h * 512 : (h + 1) * 512],
                        start=(pc == 0),
                        stop=(pc == PCH - 1),
                    )
                nc.scalar.activation(
                    out=agg_h[0:1, h * 512 : (h + 1) * 512],
                    in_=ps_a,
                    func=AF.Copy,
                    scale=coef,
                )
                nc.tensor.matmul(
                    bc_ps[:, h * 512 : (h + 1) * 512],
                    lhsT=ones_row,
                    rhs=agg_h[0:1, h * 512 : (h + 1) * 512],
                    start=True,
                    stop=True,
                )
                nc.scalar.copy(
                    out=bc_h[:, h * 512 : (h + 1) * 512],
                    in_=bc_ps[:, h * 512 : (h + 1) * 512],
                )

            # previous batch's adds/stores AFTER this chain: the DVE
            # sequencer reaches the next chain's ops before the 8-add burst
            if pending is not None:
                emit_adds(*pending)
            pending = (b, bc_h)

        for b in range(1, BLOC):
            for i in range(NT):
                load_tile(b, i)
            chain(b - 1)
            emit_q(b)
        chain(BLOC - 1)
        emit_adds(*pending)


def build_nc(repeat=1):
    nc = bacc.Bacc("TRN2", target_bir_lowering=False)
    x = nc.dram_tensor("x", [BLOC, L, D], F32, kind="ExternalInput")
    protos = nc.dram_tensor("prototypes", [P, D], F32, kind="ExternalInput")
    # fp16 output buffer: the result is computed in fp16 anyway, so storing
    # fp16 halves HBM write traffic; the host upcasts after gathering
    out = nc.dram_tensor("out", [BLOC, L, D], F16, kind="ExternalOutput")
    with tile.TileContext(nc) as tc, ExitStack() as ctx:
        _kernel(tc, ctx, x[:], protos[:], out[:], repeat=repeat)
    nc.finalize()
    return nc


def kernel(x, prototypes):
    x = np.ascontiguousarray(x, dtype=np.float32)
    prototypes = np.ascontiguousarray(prototypes, dtype=np.float32)
    assert x.shape == (B, L, D) and prototypes.shape == (P, D)
    nc = build_nc()
    in_maps = [
        {"x": x[c * BLOC : (c + 1) * BLOC], "prototypes": prototypes}
        for c in range(NCORES)
    ]
    res = run_bass_kernel_spmd(nc, in_maps, core_ids=list(range(NCORES)))
    full = np.concatenate([r["out"] for r in res.results], axis=0)
    return full.astype(np.float32)


# revision 44
# speedup vs baseline: 1.0200x; 1.0168x over previous
"""Trainium2 Bass kernel for nn_BPBookLayer (retrieval_knn).

Computation (per full input):
  query = mean(x, axis=1)                         [B, D]
  scores = cos_sim(query, prototypes)             [B, P]
  top5 -> softmax -> agg = attn @ protos[top5]    [B, D]
  out = x + 0.1 * agg[:, None, :]

Sharding: data-parallel over batch B=32 across 8 cores (4 batches/core),
prototypes replicated.

Per-core implementation notes:
 - all DMA runs through gpsimd (SWDGE) casting DMAs: x and prototypes
   load f32 HBM -> fp16 SBUF (halving SBUF-side DMA cost), the result
   stores fp16 SBUF -> f32 HBM. Loads for all 4 batches are issued
   up-front (they fit SBUF in fp16), so the DMA ring never waits on
   compute; stores queue up behind them.
 - row-space dataflow keeps the PE instruction count low (the PE
   sequencer costs ~70-135ns/instruction and drives chain latency):
     q_row[1,D]    = ones_col.T @ x_tiles      (32 matmuls/batch, 512-wide)
     qT[128,8]     = 8 outer-products against one-hot rows
     scores[1,P]   = sum_dc qT[:,dc].T @ protoT[dc]   (16 matmuls)
     wtT[128,8]    = 8 outer-products of the masked softmax row
     agg[1,D]      = sum_pc wtT[:,pc].T @ proto[pc]   (16 matmuls)
 - top-5 is selection-free: per-half DVE max8 then a merge max8; mask
   scores >= t5 and multiply by exp in one scalar_tensor_tensor.
 - prototypes live in SBUF both raw fp16 [P, D] (for agg) and
   normalized-transposed fp16 [D, P] (for scores; built on-device with
   diag(1/||p||)-scaled transpose matmuls, 4 chunks batched per PSUM
   bank to cut the PSUM->SBUF copy count).
 - residual add in-place on DVE in fp16 (2x mode): xt += bc_fp16
   broadcast; softmax/norms stay f32.
"""

from contextlib import ExitStack

import numpy as np

import concourse.bacc as bacc
import concourse.bass as bass
import concourse.tile as tile
from concourse import mybir
from concourse.bass_utils import run_bass_kernel_spmd
from concourse.masks import make_identity

F32 = mybir.dt.float32
F16 = mybir.dt.float16
AF = mybir.ActivationFunctionType
ALU = mybir.AluOpType

B, L, D, P = 32, 2048, 1024, 1024
NCORES = 8
BLOC = B // NCORES  # batches per core
TROWS = 256  # L rows per x tile
TSUB = TROWS // 128
NT = L // TROWS     # x tiles per batch
DCH = D // 128      # d chunks
PCH = P // 128      # p chunks
XBUFS = BLOC * NT   # all x tiles resident
ALPHA = 0.1


def _kernel(tc, ctx, x, protos, out, repeat=1):
    nc = tc.nc

    singles = ctx.enter_context(tc.tile_pool(name="singles", bufs=1))
    xp = ctx.enter_context(tc.tile_pool(name="xp", bufs=XBUFS))
    sm = ctx.enter_context(tc.tile_pool(name="sm", bufs=2))
    # rotating <=2KB PSUM tiles (one bank each): proto transposes, q/score/agg
    # row halves, qT/wtT outer-products
    ps4 = ctx.enter_context(tc.tile_pool(name="ps4", bufs=2, space="PSUM"))
    psq = ctx.enter_context(tc.tile_pool(name="psq", bufs=2, space="PSUM"))
    ps_bc = ctx.enter_context(tc.tile_pool(name="ps_bc", bufs=2, space="PSUM"))

    for _rep in range(repeat):
        proto_sb = singles.tile([128, PCH, D], F16)
        protoT_sb = singles.tile([128, DCH, P], F16)
        inv_pnorm = singles.tile([128, PCH], F32)
        pnorm_sq = singles.tile([128, PCH], F32)

        # ---- every load up-front on the ring: protos alternate with batch-0
        # tiles (desc-gen overlaps transfers), then all remaining x.
        # Constants (gpsimd identity fill) are emitted a few DMA pairs in so
        # the Pool engine issues the first descriptors immediately. ----
        xt = [[None] * NT for _ in range(BLOC)]

        def load_tile(b, i):
            t_ = xp.tile([128, TSUB * D], F16, tag="x", name=f"x{b}_{i}")
            xt[b][i] = t_
            nc.gpsimd.dma_start(
                out=t_,
                in_=x[b, TROWS * i : TROWS * (i + 1), :].rearrange(
                    "(p t) d -> p (t d)", p=128
                ),
            )

        ident = singles.tile([128, 128], F32)
        e8 = singles.tile([1, 8, 8], F16)
        ones_col = singles.tile([128, 1], F16)
        ones_row = singles.tile([1, 128], F16)

        # the first NHEAD prototype rows ride HWDGE in f32: their transfer
        # fills the DMA head while the first gpsimd prep runs (HWDGE
        # dispatches ~400ns sooner than SWDGE); converted to fp16 by the
        # scalar engine
        NHEAD = 36
        proto0_f32 = singles.tile([NHEAD, D], F32)
        nc.sync.dma_start(out=proto0_f32, in_=protos[0:NHEAD, :])
        nc.scalar.copy(out=proto_sb[0:NHEAD, 0, :], in_=proto0_f32)

        # SWDGE ring: batch-0 tiles alternate with proto chunks, x tile first
        # in each pair — the 1456ns tile transfer exceeds the ~1038ns/DMA prep
        # rate, so the ring self-sustains from the first transfer, and protos
        # all land by ~20us so the chain for batch 0 starts early
        # chunk-0's remainder is short (523ns) — put it last so its negative
        # prep margin lands where the ring has slack
        proto_order = list(range(1, PCH)) + [0]
        for k in range(PCH):
            c = proto_order[k]
            load_tile(0, k)
            if c == 0:
                nc.gpsimd.dma_start(
                    out=proto_sb[NHEAD:128, 0, :],
                    in_=protos[NHEAD:128, :],
                )
            else:
                nc.gpsimd.dma_start(
                    out=proto_sb[:, c, :],
                    in_=protos[c * 128 : (c + 1) * 128, :],
                )
            if k == 5:
                # ---- constants, tucked behind the queued DMA preps ----
                make_identity(nc, ident)
                nc.vector.memset(e8, 0.0)
                for j in range(8):
                    nc.vector.memset(e8[0:1, j, j : j + 1], 1.0)
                nc.vector.memset(ones_col, 1.0)
                nc.vector.memset(ones_row, 1.0)
        q_sbs, qsqs = [], []

        def emit_q(b):
            # q rows: ones.T @ x, accumulated per 512-half across all tiles;
            # tile-sem gated, so this PE work rides along with the loads and
            # stays out of the per-batch drain chain
            ps_q = [
                psq.tile([1, 512], F32, tag="q", name=f"ps_q{b}_{h}")
                for h in range(2)
            ]
            for i in range(NT):
                for t in range(TSUB):
                    for h in range(2):
                        nc.tensor.matmul(
                            ps_q[h],
                            lhsT=ones_col,
                            rhs=xt[b][i][:, t * D + h * 512 : t * D + h * 512 + 512],
                            start=(i == 0 and t == 0),
                            stop=(i == NT - 1 and t == TSUB - 1),
                        )
            q_sb = sm.tile([1, D], F16, tag="q", bufs=BLOC, name=f"q_sb{b}")
            for h in range(2):
                nc.scalar.copy(out=q_sb[0:1, h * 512 : (h + 1) * 512], in_=ps_q[h])
            qsq_sc = sm.tile([1, 512], F32, tag="qsq_sc")
            qsq = sm.tile([1, 2], F32, tag="qsq", bufs=BLOC, name=f"qsq{b}")
            for h in range(2):
                nc.scalar.activation(
                    out=qsq_sc, in_=ps_q[h], func=AF.Square,
                    accum_out=qsq[0:1, h : h + 1],
                )
            q_sbs.append(q_sb)
            qsqs.append(qsq)

        emit_q(0)

        # ---- prototype norms + normalized transpose (off the load ring) ----
        sq_scratch = sm.tile([128, D], F32, tag="sqs", bufs=1)
        for c in proto_order:
            nc.scalar.activation(
                out=sq_scratch,
                in_=proto_sb[:, c, :],
                func=AF.Square,
                accum_out=pnorm_sq[:, c : c + 1],
            )
            nc.scalar.activation(
                out=inv_pnorm[:, c : c + 1], in_=pnorm_sq[:, c : c + 1], func=AF.Sqrt
            )
            nc.vector.reciprocal(
                out=inv_pnorm[:, c : c + 1], in_=inv_pnorm[:, c : c + 1]
            )
            # protoT_n[d, p] = proto[p, d] / ||proto_p||  via lhsT.T @ diag;
            # 4 transposed chunks share one PSUM bank (start zeroes the bank,
            # later chunks accumulate into their own zeroed columns)
            diag_c = sm.tile([128, 128], F16, tag="diag", bufs=2, name=f"diag_{c}")
            nc.vector.tensor_scalar_mul(diag_c, ident, inv_pnorm[:, c : c + 1])
            for half in range(2):
                pst = ps4.tile([128, 4, 128], F32, tag="ps")
                for j in range(4):
                    dc = half * 4 + j
                    nc.tensor.matmul(
                        pst[:, j, :],
                        lhsT=proto_sb[:, c, dc * 128 : (dc + 1) * 128],
                        rhs=diag_c,
                        start=(j == 0),
                        stop=(j == 3),
                        skip_group_check=True,
                    )
                dst = protoT_sb[:, half * 4 : half * 4 + 4, c * 128 : (c + 1) * 128]
                if half == 0:
                    nc.scalar.copy(out=dst, in_=pst)
                else:
                    nc.vector.tensor_copy(dst, pst)

        # ---- per batch chains (q already computed in the load phase) ----
        def emit_adds(b, bc_h):
            bc_b = bc_h.rearrange("p (o d) -> p o d", o=1).to_broadcast(
                [128, TSUB, D]
            )
            bc_1 = bc_h.rearrange("p (o d) -> p o d", o=1).to_broadcast(
                [128, 1, D]
            )
            for i in range(NT):
                xv = xt[b][i].rearrange("p (t d) -> p t d", d=D)
                if i == 0:
                    # first tile split into quarters: its first store launches
                    # ~800ns sooner, shrinking every batch-boundary DMA gap
                    hbm = out[b, TROWS * i : TROWS * (i + 1), :].rearrange(
                        "(p t) d -> p t d", p=128
                    )
                    for t in range(TSUB):
                        for hh in range(2):
                            sl = slice(hh * 512, hh * 512 + 512)
                            bc_q = bc_h[:, sl].rearrange("p (o d) -> p o d", o=1)
                            nc.vector.tensor_tensor(
                                out=xv[:, t : t + 1, sl],
                                in0=xv[:, t : t + 1, sl],
                                in1=bc_q.to_broadcast([128, 1, 512]),
                                op=ALU.add,
                            )
                            nc.sync.dma_start(
                                out=hbm[:, t, sl],
                                in_=xt[b][i][:, t * D + hh * 512 : t * D + hh * 512 + 512],
                            )
                    continue
                nc.vector.tensor_tensor(out=xv, in0=xv, in1=bc_b, op=ALU.add)
                nc.sync.dma_start(
                    out=out[b, TROWS * i : TROWS * (i + 1), :].rearrange(
                        "(p t) d -> p (t d)", p=128
                    ),
                    in_=xt[b][i],
                )

        pending = None

        def chain(b):
            nonlocal pending
            q_sb, qsq = q_sbs[b], qsqs[b]
            inv_qn = sm.tile([1, 1], F32, tag="inv_qn")
            nc.vector.tensor_add(inv_qn, qsq[0:1, 0:1], qsq[0:1, 1:2])
            nc.scalar.activation(out=inv_qn, in_=inv_qn, func=AF.Sqrt)
            nc.vector.reciprocal(out=inv_qn, in_=inv_qn)

            # qT[128, 8] via one-hot outer products (one PSUM group)
            ps_qt = ps4.tile([128, DCH], F32, tag="ps")
            for dc in range(DCH):
                nc.tensor.matmul(
                    ps_qt,
                    lhsT=q_sb[0:1, dc * 128 : (dc + 1) * 128],
                    rhs=e8[0:1, dc, :],
                    start=(dc == 0),
                    stop=(dc == DCH - 1),
                )
            qT_h = sm.tile([128, DCH], F16, tag="qTh")
            nc.scalar.copy(out=qT_h, in_=ps_qt)

            # scores row: sum_dc qT[:,dc].T @ protoT_n[dc]
            ps_s = [ps4.tile([1, 512], F32, tag="ps", name=f"ps_s{h}") for h in range(2)]
            for dc in range(DCH):
                for h in range(2):
                    nc.tensor.matmul(
                        ps_s[h],
                        lhsT=qT_h[:, dc : dc + 1],
                        rhs=protoT_sb[:, dc, h * 512 : (h + 1) * 512],
                        start=(dc == 0),
                        stop=(dc == DCH - 1),
                    )

            # top-8 per half, then merged top-8; t5 = 5th largest overall
            vals2 = sm.tile([1, 16], F32, tag="vals2")
            for h in range(2):
                nc.vector.max(out=vals2[0:1, 8 * h : 8 * h + 8], in_=ps_s[h])
            vals = sm.tile([1, 8], F32, tag="vals")
            nc.vector.max(out=vals, in_=vals2)

            # e = exp(scores / ||q||); den over the top-5; coef = 0.1/den
            e_row = sm.tile([1, P], F32, tag="erow")
            for h in range(2):
                nc.scalar.activation(
                    out=e_row[0:1, h * 512 : (h + 1) * 512],
                    in_=ps_s[h],
                    func=AF.Exp,
                    scale=inv_qn,
                )
            evals = sm.tile([1, 8], F32, tag="evals")
            nc.scalar.activation(out=evals, in_=vals, func=AF.Exp, scale=inv_qn)
            den = sm.tile([1, 1], F32, tag="den")
            nc.vector.reduce_sum(
                out=den, in_=evals[0:1, 0:5], axis=mybir.AxisListType.X
            )
            coef = sm.tile([1, 1], F32, tag="coef")
            nc.vector.reciprocal(out=coef, in_=den)
            nc.scalar.mul(out=coef, in_=coef, mul=ALPHA)

            # wt row = (scores >= t5) * e, fp16
            wt_h = sm.tile([1, P], F16, tag="wth")
            for h in range(2):
                nc.vector.scalar_tensor_tensor(
                    out=wt_h[0:1, h * 512 : (h + 1) * 512],
                    in0=ps_s[h],
                    scalar=vals[0:1, 4:5],
                    in1=e_row[0:1, h * 512 : (h + 1) * 512],
                    op0=ALU.is_ge,
                    op1=ALU.mult,
                )

            # wtT[128, 8] via one-hot outer products
            ps_wt = ps4.tile([128, PCH], F32, tag="ps")
            for pc in range(PCH):
                nc.tensor.matmul(
                    ps_wt,
                    lhsT=wt_h[0:1, pc * 128 : (pc + 1) * 128],
                    rhs=e8[0:1, pc, :],
                    start=(pc == 0),
                    stop=(pc == PCH - 1),
                )
            wtT_h = sm.tile([128, PCH], F16, tag="wtTh")
            nc.scalar.copy(out=wtT_h, in_=ps_wt)

            # agg row: sum_pc wtT[:,pc].T @ proto[pc], scaled into fp16
            # agg -> scale-copy -> broadcast -> fp16 copy, pipelined per
            # 512-half: half 0's Act/PE tail overlaps half 1's aggregation
            agg_h = sm.tile([1, D], F16, tag="agg")
            bc_ps = ps_bc.tile([128, D], F32, tag="bc")
            bc_h = sm.tile([128, D], F16, tag="bch")
            for h in range(2):
                ps_a = ps4.tile([1, 512], F32, tag="ps", name=f"ps_a{h}")
                for pc in range(PCH):
                    nc.tensor.matmul(
                        ps_a,
                        lhsT=wtT_h[:, pc : pc + 1],
                        rhs=proto_sb[:, pc, h * 512 : (h + 1) * 512],
                        start=(pc == 0),
                        stop=(pc == PCH - 1),
                    )
                nc.scalar.activation(
                    out=agg_h[0:1, h * 512 : (h + 1) * 512],
                    in_=ps_a,
                    func=AF.Copy,
                    scale=coef,
                )
                nc.tensor.matmul(
                    bc_ps[:, h * 512 : (h + 1) * 512],
                    lhsT=ones_row,
                    rhs=agg_h[0:1, h * 512 : (h + 1) * 512],
                    start=True,
                    stop=True,
                )
                nc.scalar.copy(
                    out=bc_h[:, h * 512 : (h + 1) * 512],
                    in_=bc_ps[:, h * 512 : (h + 1) * 512],
                )

            # previous batch's adds/stores AFTER this chain: the DVE
            # sequencer reaches the next chain's ops before the 8-add burst
            if pending is not None:
                emit_adds(*pending)
            pending = (b, bc_h)

        for b in range(1, BLOC):
            for i in range(NT):
                load_tile(b, i)
            chain(b - 1)
            emit_q(b)
        chain(BLOC - 1)
        emit_adds(*pending)


def build_nc(repeat=1):
    nc = bacc.Bacc("TRN2", target_bir_lowering=False)
    x = nc.dram_tensor("x", [BLOC, L, D], F32, kind="ExternalInput")
    protos = nc.dram_tensor("prototypes", [P, D], F32, kind="ExternalInput")
    # fp16 output buffer: the result is computed in fp16 anyway, so storing
    # fp16 halves HBM write traffic; the host upcasts after gathering
    out = nc.dram_tensor("out", [BLOC, L, D], F16, kind="ExternalOutput")
    with tile.TileContext(nc) as tc, ExitStack() as ctx:
        _kernel(tc, ctx, x[:], protos[:], out[:], repeat=repeat)
    nc.finalize()
    return nc


def kernel(x, prototypes):
    x = np.ascontiguousarray(x, dtype=np.float32)
    prototypes = np.ascontiguousarray(prototypes, dtype=np.float32)
    assert x.shape == (B, L, D) and prototypes.shape == (P, D)
    nc = build_nc()
    in_maps = [
        {"x": x[c * BLOC : (c + 1) * BLOC], "prototypes": prototypes}
        for c in range(NCORES)
    ]
    res = run_bass_kernel_spmd(nc, in_maps, core_ids=list(range(NCORES)))
    full = np.concatenate([r["out"] for r in res.results], axis=0)
    return full.astype(np.float32)
